# revision 14
# baseline (speedup 1.0000x reference)
"""Viterbi CRF decode kernel for Trainium2 (Bass), 8-core data parallel.

reference (per batch b):
    score = pot[b,0,:]
    for t in 1..L-1:
        cand[i,j] = score[i] + A[i,j]
        bp_t[j]   = argmax_i cand (first occurrence)   (identity if t>=len)
        score     = max_i cand + pot[b,t,:]            (frozen if t>=len)
    last = argmax_j score ; backtrace through bp -> tags [L]
returns (tags int32 [B,L], transition_params passthrough)

Sharding: batch 256 -> 8 cores x 32 (pure data parallel).

Per-core layout: partition p = 4*b + jg (b<32, jg<4); state j = 12*jg + jj.
cand free dims = (jj=12, i=48). The i-max lives entirely in the free dim, so
tensor_reduce(X) gives the full per-j max. The new score [128,12] is
j-sharded across partitions; a PE matmul against constant 0/1 selector
matrices re-replicates it into a [128,48] "score_rep" PSUM tile and a second
selector matmul accumulates pot[:,t,:] into it. Backpointers are stored in
reversed-index space (bpr = 47 - argmax_i, extracted by eq/iota/reduce with
first-occurrence tie semantics). The backtrace walk does one
scalar_tensor_tensor gather per step plus a PE selector matmul to sum the 4
per-partition partials; tags stay in reversed space and the host computes
47 - rtag at the end.
"""

import os
import sys

for _p in ("/opt/trn_rl_repo", "/root/.axon_site/_ro/trn_rl_repo"):
    if os.path.isdir(_p) and _p not in sys.path:
        sys.path.insert(0, _p)

import numpy as np

import concourse.bass as bass
import concourse.mybir as mybir

F32 = mybir.dt.float32
AX = mybir.AxisListType
OP = mybir.AluOpType
ET = mybir.EngineType

B, L, T = 256, 2048, 48
NCORES = 8
BC = B // NCORES          # 32 batch per core
JG = 4                    # j-groups per batch
JJ = T // JG              # 12 states per partition
P = BC * JG               # 128 partitions
CHUNK = 128               # pot timesteps per DMA chunk
FREE = JJ * T             # 576 cand elements per partition


# ---------------------------------------------------------------- constants
def host_constants():
    i_arr = np.arange(T, dtype=np.float32)
    iotar = np.tile(47.0 - i_arr, JJ)[None, :].repeat(P, axis=0).astype(np.float32)
    jidxr = np.zeros((P, JJ), np.float32)
    for jg in range(JG):
        jidxr[jg::JG, :] = 47.0 - (JJ * jg + np.arange(JJ, dtype=np.float32))[None, :]
    iotar48 = np.tile((47.0 - i_arr)[None, :], (P, 1)).astype(np.float32)
    bsel = np.zeros((P, JG * P), np.float32)
    for g in range(JG):
        for m in range(P):
            bsel[(m // JG) * JG + g, g * P + m] = 1.0
    rep4 = np.zeros((BC, P), np.float32)
    for m in range(P):
        rep4[m // JG, m] = 1.0
    kk = np.arange(P)
    wsel = (kk[:, None] // JG == kk[None, :] // JG).astype(np.float32)
    return iotar, jidxr, iotar48, bsel, rep4, wsel


def at_rep_from_A(A: np.ndarray) -> np.ndarray:
    at = np.zeros((P, FREE), np.float32)
    for jg in range(JG):
        blk = A.T[JJ * jg : JJ * (jg + 1), :].reshape(FREE)  # (jj, i) flat
        at[jg::JG, :] = blk[None, :]
    return at


# ---------------------------------------------------------------- kernel
def build_crf_kernel(n_steps: int = L, general: bool = False) -> bass.Bass:
    assert n_steps % CHUNK == 0
    nchunks = n_steps // CHUNK
    ns1 = n_steps - 1  # number of recursion steps (t = 1 .. n_steps-1)
    nc = bass.Bass()

    pot = nc.dram_tensor("pot", [BC, n_steps, T], F32, kind="ExternalInput")
    at_rep_d = nc.dram_tensor("at_rep", [P, FREE], F32, kind="ExternalInput")
    iotar_d = nc.dram_tensor("iotar", [P, FREE], F32, kind="ExternalInput")
    jidxr_d = nc.dram_tensor("jidxr", [P, JJ], F32, kind="ExternalInput")
    iotar48_d = nc.dram_tensor("iotar48", [P, T], F32, kind="ExternalInput")
    bsel_d = nc.dram_tensor("bsel", [P, JG * P], F32, kind="ExternalInput")
    rep4_d = nc.dram_tensor("rep4", [BC, P], F32, kind="ExternalInput")
    wsel_d = nc.dram_tensor("wsel", [P, P], F32, kind="ExternalInput")
    if general:
        vsel_d = nc.dram_tensor("vsel", [P, n_steps], F32, kind="ExternalInput")
        ivsel_d = nc.dram_tensor("ivsel", [P, n_steps], F32, kind="ExternalInput")
        vbp_d = nc.dram_tensor("vbp", [P, n_steps], F32, kind="ExternalInput")
        ivbp_d = nc.dram_tensor("ivbp", [P, n_steps], F32, kind="ExternalInput")
    rtags_out = nc.dram_tensor("rtags", [BC, n_steps], F32, kind="ExternalOutput")

    from contextlib import ExitStack

    with ExitStack() as ctx:
        sb = lambda name, shape: ctx.enter_context(nc.sbuf_tensor(name, shape, F32))
        at_rep_sb = sb("at_rep_sb", [P, FREE])
        iotar_sb = sb("iotar_sb", [P, FREE])
        jidxr_sb = sb("jidxr_sb", [P, JJ])
        iotar48_sb = sb("iotar48_sb", [P, T])
        bsel_sb = sb("bsel_sb", [P, JG * P])
        rep4_sb = sb("rep4_sb", [BC, P])
        wsel_sb = sb("wsel_sb", [P, P])
        pot_sb = sb("pot_sb", [BC, 2 * CHUNK * T])
        cand_sb = sb("cand_sb", [P, FREE])
        eqm_sb = sb("eqm_sb", [P, FREE])
        pmax_sb = sb("pmax_sb", [P, JJ])
        bp_sb = sb("bp_sb", [P, ns1 * JJ])
        tags_sb = sb("tags_sb", [P, n_steps])
        wpart_sb = sb("wpart_sb", [P, 1])
        wscr_sb = sb("wscr_sb", [P, JJ])
        maxv_sb = sb("maxv_sb", [P, 1])
        eqf_sb = sb("eqf_sb", [P, T])
        vsel_sb = sb("vsel_sb", [P, n_steps if general else 1])
        ivsel_sb = sb("ivsel_sb", [P, n_steps if general else 1])
        vbp_sb = sb("vbp_sb", [P, n_steps if general else 1])
        ivbp_sb = sb("ivbp_sb", [P, n_steps if general else 1])
        olds_sb = sb("olds_sb", [P, T if general else 1])
        selu_sb = sb("selu_sb", [P, T if general else 1])
        bpr_sb = sb("bpr_sb", [P, JJ if general else 1])
        bpu_sb = sb("bpu_sb", [P, JJ if general else 1])
        score_ps = ctx.enter_context(nc.psum_tensor("score_ps", [P, T], F32))
        tag_ps = ctx.enter_context(nc.psum_tensor("tag_ps", [P, 1], F32))
        sem = lambda name: ctx.enter_context(nc.semaphore(name))
        s_init = sem("s_init")
        s_potin = sem("s_potin")
        s_potdone = sem("s_potdone")
        s_pmax = sem("s_pmax")
        s_score = sem("s_score")
        s_w1 = sem("s_w1")
        s_w2 = sem("s_w2")
        s_out = sem("s_out")
        s_dout = sem("s_dout")
        def sps():  # [P, T] view of the score psum bank
            return bass.AP(score_ps, 0, [[T, P], [1, T]])

        def sps_bcast():
            return bass.AP(score_ps, 0, [[T, P], [0, JJ], [1, T]])

        olds_ap = bass.AP(olds_sb, 0, [[T, P], [1, T]]) if general else None
        olds_bc = bass.AP(olds_sb, 0, [[T, P], [0, JJ], [1, T]]) if general else None

        cand3 = bass.AP(cand_sb, 0, [[FREE, P], [T, JJ], [1, T]])
        pmax_bcast = bass.AP(pmax_sb, 0, [[JJ, P], [1, JJ], [0, T]])

        # ---------------- prologue: constant DMAs (SP) ----------------
        const_pairs = [
            (at_rep_sb, at_rep_d),
            (iotar_sb, iotar_d),
            (jidxr_sb, jidxr_d),
            (iotar48_sb, iotar48_d),
            (bsel_sb, bsel_d),
            (rep4_sb, rep4_d),
            (wsel_sb, wsel_d),
        ]
        if general:
            const_pairs += [(vsel_sb, vsel_d), (ivsel_sb, ivsel_d),
                            (vbp_sb, vbp_d), (ivbp_sb, ivbp_d)]
        for sb, d in const_pairs:
            nc.sync.dma_start(sb[:, :], d[:, :]).then_inc(s_init, 16)
        n_const = len(const_pairs)
        # pot chunk 0 -> buffer 0
        nc.sync.dma_start(
            pot_sb[:, 0 : CHUNK * T], pot[:, 0:CHUNK, :]
        ).then_inc(s_potin, 16)

        # ---------------- PE prologue: raw score_0 ----------------
        nc.tensor.wait_ge(s_init, 16 * n_const)
        nc.tensor.wait_ge(s_potin, 16)
        nc.tensor.matmul(
            sps(),
            rep4_sb[:, :],
            pot_sb[:, 0:T],
            start=True,
            stop=True,
        ).then_inc(s_score, 1)

        nc.vector.wait_ge(s_init, 16 * n_const)
        if general:
            nc.vector.memset(olds_sb[:, :], 0.0)
            nc.vector.drain()

        # ---------------- main loop: chunk 0 python-emitted, rest nested ----------------
        def emit_fwd_body(t, pot_off):
            """Forward-step body; t is loop register, pot_off an offset expr
            such that pot_sb[:, pot_off : pot_off+T] is pot[:, t, :]."""
            bp_off = nc.s_assert_within(
                (t - 1) * JJ, 0, (ns1 - 1) * JJ, skip_runtime_assert=True
            )
            nc.vector.wait_ge(s_score, t)
            if general:
                nc.vector.tensor_scalar(
                    selu_sb[:, :],
                    sps(),
                    vsel_sb[:, bass.ds(t, 1)],
                    None,
                    OP.mult,
                )
                nc.vector.drain()
                nc.vector.scalar_tensor_tensor(
                    out=olds_ap,
                    in0=olds_ap,
                    scalar=ivsel_sb[:, bass.ds(t, 1)],
                    in1=selu_sb[:, :],
                    op0=OP.mult,
                    op1=OP.add,
                )
                nc.vector.drain()
                score_src = olds_bc
            else:
                score_src = sps_bcast()

            # cand = score_{t-1} (bcast over jj) + at_rep
            nc.vector.tensor_tensor(cand_sb[:, :], score_src, at_rep_sb[:, :], OP.add)
            nc.vector.drain()
            # pmax over i
            nc.vector.tensor_reduce(
                pmax_sb[:, :], cand3, AX.X, OP.max
            ).then_inc(s_pmax, 1)
            nc.vector.drain()

            # ---- PE: regather pmax + pot[t] -> raw score_t ----
            nc.tensor.wait_ge(s_pmax, t)
            for g in range(JG):
                nc.tensor.matmul(
                    bass.AP(score_ps, g * JJ, [[T, P], [1, JJ]]),
                    bsel_sb[:, g * P : (g + 1) * P],
                    pmax_sb[:, :],
                    start=(g == 0),
                    stop=False,
                    skip_group_check=True,
                )
            nc.tensor.matmul(
                sps(),
                rep4_sb[:, :],
                pot_sb[:, bass.ds(pot_off, T)],
                start=False,
                stop=True,
                skip_group_check=True,
            ).then_inc(s_score, 1)

            # ---- DVE: backpointer extraction (overlaps PE) ----
            nc.vector.tensor_tensor(eqm_sb[:, :], cand_sb[:, :], pmax_bcast, OP.is_equal)
            nc.vector.drain()
            nc.vector.tensor_tensor(cand_sb[:, :], eqm_sb[:, :], iotar_sb[:, :], OP.mult)
            nc.vector.drain()
            if general:
                nc.vector.tensor_reduce(bpr_sb[:, :], cand3, AX.X, OP.max)
                nc.vector.drain()
                nc.vector.tensor_scalar(
                    bpu_sb[:, :],
                    bpr_sb[:, :],
                    vbp_sb[:, bass.ds(t, 1)],
                    None,
                    OP.mult,
                )
                nc.vector.drain()
                nc.vector.scalar_tensor_tensor(
                    out=bp_sb[:, bass.ds(bp_off, JJ)],
                    in0=jidxr_sb[:, :],
                    scalar=ivbp_sb[:, bass.ds(t, 1)],
                    in1=bpu_sb[:, :],
                    op0=OP.mult,
                    op1=OP.add,
                )
            else:
                nc.vector.tensor_reduce(
                    bp_sb[:, bass.ds(bp_off, JJ)], cand3, AX.X, OP.max
                )
            nc.vector.drain()

        # SP: prefetch chunk 1 now (prologue already fetched chunk 0)
        if nchunks > 1:
            nc.sync.wait_ge(s_potin, 16)
            nc.sync.dma_start(
                pot_sb[:, CHUNK * T : 2 * CHUNK * T],
                pot[:, CHUNK : 2 * CHUNK, :],
            ).then_inc(s_potin, 16)
        # GPSIMD: remaining prefetches, pair-unrolled so the SBUF-side DMA
        # offsets stay static (dynamic SBUF APs need bacc); chunk c -> buf c%2
        if nchunks > 2:
            assert nchunks % 2 == 0
            with nc.Fori(0, (nchunks - 2) // 2, engines=[ET.Pool]) as q:
                for half in (0, 1):
                    cc = 2 * q + 2 + half  # chunk index expr
                    nc.gpsimd.wait_ge(s_potdone, cc - 1)
                    nc.gpsimd.wait_ge(s_potin, 16 * cc)
                    nc.gpsimd.dma_start(
                        pot_sb[:, half * CHUNK * T : (half + 1) * CHUNK * T],
                        pot[:, bass.ds(cc * CHUNK, CHUNK), :],
                    ).then_inc(s_potin, 16)

        # chunks 0..nchunks-1 ; chunk 0 starts at t=1 via the arithmetic bound
        with nc.Fori(0, nchunks, engines=[ET.PE, ET.DVE]) as c:
            nc.tensor.wait_ge(s_potin, 16 * (c + 1))
            t_start = c * CHUNK + 1 - (c + nchunks - 1) // nchunks
            with nc.Fori(t_start, c * CHUNK + CHUNK, engines=[ET.PE, ET.DVE]) as t:
                pot_off = nc.s_assert_within(
                    t * T + ((c % 2) - c) * (CHUNK * T),
                    0,
                    2 * CHUNK * T - T,
                    skip_runtime_assert=True,
                )
                emit_fwd_body(t, pot_off)
            nc.tensor.sem_inc(s_potdone, 1)

        # ---------------- epilogue: final score + init rtag ----------------
        nc.vector.wait_ge(s_score, n_steps)
        nc.vector.drain()
        if general:
            # select raw_{ns1} -> olds[(ns1-? ) ]: writes olds[(ns1+1)%2] pattern
            nc.vector.tensor_scalar(
                selu_sb[:, :],
                sps(),
                vbp_sb[:, ns1 : ns1 + 1],
                None,
                OP.mult,
            )
            nc.vector.drain()
            nc.vector.scalar_tensor_tensor(
                out=olds_ap,
                in0=olds_ap,
                scalar=ivbp_sb[:, ns1 : ns1 + 1],
                in1=selu_sb[:, :],
                op0=OP.mult,
                op1=OP.add,
            )
            nc.vector.drain()
            fscore = olds_ap
        else:
            fscore = sps()
        nc.vector.tensor_reduce(maxv_sb[:, :], fscore, AX.X, OP.max)
        nc.vector.drain()
        nc.vector.tensor_scalar(
            eqf_sb[:, :], fscore, maxv_sb[:, 0:1], None, OP.is_equal
        )
        nc.vector.drain()
        nc.vector.tensor_tensor(cand_sb[:, 0:T], eqf_sb[:, :], iotar48_sb[:, :], OP.mult)
        nc.vector.drain()
        nc.vector.tensor_reduce(
            tags_sb[:, ns1:n_steps],
            bass.AP(cand_sb, 0, [[FREE, P], [1, T]]),
            AX.X,
            OP.max,
        )

        # ---------------- backtrace walk ----------------
        nc.vector.drain()
        with nc.Fori(0, ns1, engines=[ET.PE, ET.DVE]) as w:
            nc.vector.scalar_tensor_tensor(
                out=wscr_sb[:, :],
                in0=jidxr_sb[:, :],
                scalar=tags_sb[:, bass.ds(ns1 - w, 1)],
                in1=bp_sb[:, bass.ds((ns1 - 1 - w) * JJ, JJ)],
                op0=OP.is_equal,
                op1=OP.mult,
                accum_out=wpart_sb[:, :],
            ).then_inc(s_w1, 1)
            nc.tensor.wait_ge(s_w1, w + 1)
            nc.tensor.matmul(
                tag_ps[:, :], wsel_sb[:, :], wpart_sb[:, :], start=True, stop=True
            ).then_inc(s_w2, 1)
            nc.vector.wait_ge(s_w2, w + 1)
            nc.vector.tensor_copy(
                tags_sb[:, bass.ds(ns1 - 1 - w, 1)], tag_ps[:, :]
            ).then_inc(s_out, 1)
            nc.vector.drain()

        nc.sync.wait_ge(s_out, ns1)
        nc.sync.dma_start(
            rtags_out[:, :],
            bass.AP(tags_sb, 0, [[JG * n_steps, BC], [1, n_steps]]),
        ).then_inc(s_dout, 16)
        nc.sync.wait_ge(s_dout, 16)

    return nc


# ---------------------------------------------------------------- host API
_CACHE: dict = {}


def _get_kernel(n_steps: int, general: bool) -> bass.Bass:
    key = (n_steps, general)
    if key not in _CACHE:
        _CACHE[key] = build_crf_kernel(n_steps, general)
    return _CACHE[key]


def make_in_maps(potentials, lengths, transition_params, n_steps=L, general=False):
    iotar, jidxr, iotar48, bsel, rep4, wsel = host_constants()
    at_rep = at_rep_from_A(np.asarray(transition_params, np.float32))
    ncores = potentials.shape[0] // BC
    in_maps = []
    for c in range(ncores):
        m = {
            "pot": np.ascontiguousarray(potentials[c * BC : (c + 1) * BC], np.float32),
            "at_rep": at_rep,
            "iotar": iotar,
            "jidxr": jidxr,
            "iotar48": iotar48,
            "bsel": bsel,
            "rep4": rep4,
            "wsel": wsel,
        }
        if general:
            ln = np.asarray(lengths[c * BC : (c + 1) * BC], np.int64)
            steps = np.arange(n_steps, dtype=np.int64)
            v = (steps[None, :] < ln[:, None]).astype(np.float32)  # valid_s
            v[:, 0] = 1.0  # step 0 (init score) is always taken
            # vsel[t] = valid_{t-1} (select of raw_{t-1} at iteration t)
            vsel = np.concatenate([np.ones((BC, 1), np.float32), v[:, :-1]], axis=1)
            vbp = v  # vbp[t] = valid_t (bp select + final-raw select)
            m["vsel"] = np.ascontiguousarray(np.repeat(vsel, JG, axis=0))
            m["ivsel"] = np.ascontiguousarray(1.0 - m["vsel"])
            m["vbp"] = np.ascontiguousarray(np.repeat(vbp, JG, axis=0))
            m["ivbp"] = np.ascontiguousarray(1.0 - m["vbp"])
        in_maps.append(m)
    return in_maps


def kernel(potentials, lengths, transition_params):
    from concourse import bass_utils

    potentials = np.asarray(potentials, np.float32)
    lengths = np.asarray(lengths, np.int32)
    transition_params = np.asarray(transition_params, np.float32)

    general = not bool(np.all(lengths >= L))
    nc = _get_kernel(L, general)
    in_maps = make_in_maps(potentials, lengths, transition_params, L, general)
    res = bass_utils.run_bass_kernel_spmd(nc, in_maps, core_ids=list(range(NCORES)))
    outs = res.results
    rtags = np.concatenate([o["rtags"] for o in outs], axis=0)  # [B, L] f32
    tags = (47.0 - rtags).astype(np.int32)
    return tags, transition_params
